# revision 1
# baseline (speedup 1.0000x reference)
"""BertSelfAttention forward on 8 Trainium2 NeuronCores (Bass/Tile).

Problem: B=4, S=2048, D=1024, H=16 heads, head_dim=64.
Sharding: 8 cores = (batch b in 0..4) x (head-group g in 0..2); each core
computes heads [8g, 8g+8) of batch b => output slice out[b, :, 512g:512(g+1)].

Per-core kernel (all matmuls in float32r: full PE rate, ~2e-4 rel err):
  phase 1 (emission interleaved so attention can start as data lands):
    qT = (wq @ x.T) + bq      [dh=512, S]  (dh on partitions)
    kT = (wk @ x.T) + bk      [dh=512, S]
    v  = (x @ wv.T + bv)*pad  [S, dh=512]  stored per s-tile as
         [128, 8 heads, 65] with col 64 = pad column -> the PV matmul
         emits softmax denominators for free (M=65).
  phase 2: per head-pair (2p, 2p+1): the two heads' score matmuls are
    row-tiled on the PE (K=64 each at array rows 0:64 / 64:128) and run
    CONCURRENTLY (~116ns/MM measured); both write one [128, 1024] PSUM
    group (2 banks, double-buffered) so a single ScalarE activation does
    exp(0.125*scores) for both heads -> e[ks, 0:512]=head A, [512:1024]=B.
    ctxT_aug[65, qs] += v_aug.T @ e  accumulated over 16 k-tiles; then
    PE-transpose 128-blocks, divide by the denominator column
    (VectorE reciprocal + tensor_scalar), DMA out.

The pad mask is exact: exp(s + (1-pad)*-1e9) = exp(s)*pad, folded into the
v rows and the denominator column (both scaled by pad in phase 1).
"""

import numpy as np

import concourse.bass as bass
import concourse.tile as tile
from concourse import mybir
from concourse.bass import ds, ts
from concourse.masks import make_identity

B, S, D, H = 4, 2048, 1024, 16
HD = D // H          # 64
DH = 512             # head dims per core (8 heads)
NHEADS = 8           # heads per core
NPAIRS = 4           # head pairs per core
KT = 16              # k-tiles of 128 over S
QC = 4               # q-chunks of 512 over S
KD = 8               # contraction tiles of 128 over D

F32 = mybir.dt.float32
F32R = mybir.dt.float32r

MAX_WAITS = 1


def split_excess_waits(nc):
    """This walrus build accepts only one sync-wait per instruction; hoist
    excess waits onto preceding NoOps on the same engine."""
    for f in nc.m.functions:
        for bb in f.blocks:
            insts = list(bb.instructions)
            out = []
            changed = False
            for inst in insts:
                si = inst.sync_info
                if si is not None and si.on_wait and len(si.on_wait) > MAX_WAITS:
                    waits = list(si.on_wait)
                    changed = True
                    k = 0
                    while len(waits) - k > MAX_WAITS:
                        nop = mybir.InstNoOp(
                            name=f"{inst.name}-ws{k}", engine=inst.engine
                        )
                        nop.sync_info = mybir.SyncInfo(
                            on_wait=waits[k : k + MAX_WAITS], on_update=[]
                        )
                        out.append(nop)
                        k += MAX_WAITS
                    si.on_wait = waits[k:]
                out.append(inst)
            if changed:
                bb.instructions = out


def build_nc():
    nc = bass.Bass("TRN2", target_bir_lowering=False, debug=False, num_devices=8)

    xT_d = nc.declare_dram_parameter("xT", [D, S], F32R, isOutput=False)
    wqT_d = nc.declare_dram_parameter("wqT", [D, DH], F32R, isOutput=False)
    wkT_d = nc.declare_dram_parameter("wkT", [D, DH], F32R, isOutput=False)
    wvT_d = nc.declare_dram_parameter("wvT", [D, DH], F32R, isOutput=False)
    bq_d = nc.declare_dram_parameter("bq", [DH], F32, isOutput=False)
    bk_d = nc.declare_dram_parameter("bk", [DH], F32, isOutput=False)
    bv_d = nc.declare_dram_parameter("bv", [DH], F32, isOutput=False)
    pad_d = nc.declare_dram_parameter("pad", [S], F32, isOutput=False)
    out_d = nc.declare_dram_parameter("ctx", [S, DH], F32, isOutput=True)

    with tile.TileContext(nc) as tc:
        with (
            tc.tile_pool(name="persist", bufs=1) as persist,
            tc.tile_pool(name="small", bufs=1) as small,
            tc.tile_pool(name="phX", bufs=1) as phX,
        ):
            qT_sb = persist.tile([128, 4, S], F32R)
            kT_sb = persist.tile([128, 4, S], F32R)
            v_sb = persist.tile([128, KT, NHEADS, HD + 1], F32R)
            bq_sb = small.tile([128, 4], F32)
            bk_sb = small.tile([128, 4], F32)
            pad_sb = small.tile([128, KT], F32)
            bv_bc = small.tile([128, DH], F32)
            ident = small.tile([128, 128], F32)

            nc.gpsimd.dma_start(out=bq_sb, in_=bq_d.ap().rearrange("(m p) -> p m", p=128))
            nc.gpsimd.dma_start(out=bk_sb, in_=bk_d.ap().rearrange("(m p) -> p m", p=128))
            nc.gpsimd.dma_start(out=pad_sb, in_=pad_d.ap().rearrange("(t p) -> p t", p=128))
            nc.gpsimd.dma_start(
                out=bv_bc,
                in_=bass.AP(tensor=bv_d, offset=0, ap=[[0, 128], [1, DH]]),
            )
            make_identity(nc, ident[:])
            # whole-tile memset: the v evac writes cols 0:64 of each head slot,
            # leaving col 64 = 1.0 (denominator column), then *= pad
            nc.vector.memset(v_sb[:].bitcast(F32), 1.0)

            xT_r = xT_d.ap().rearrange("(t p) s -> p t s", p=128)
            wq_r = wqT_d.ap().rearrange("(t p) n -> p t n", p=128)
            wk_r = wkT_d.ap().rearrange("(t p) n -> p t n", p=128)
            wv_r = wvT_d.ap().rearrange("(t p) n -> p t n", p=128)

            xT_sb = phX.tile([128, KD, S], F32R)

            # ---------------- phase 1a: v projection ----------------
            with (
                tc.tile_pool(name="phV", bufs=1) as phV,
                tc.tile_pool(name="ps1", bufs=4, space="PSUM") as ps1,
            ):
                wv_sb = phV.tile([128, KD, DH], F32R)
                # kd-split DMAs so accumulation starts while data streams in;
                # xT on the sync engine (HWDGE) in parallel with gpsimd
                for kd in range(KD):
                    nc.sync.dma_start(out=xT_sb[:, kd, :], in_=xT_r[:, kd, :])
                    nc.gpsimd.dma_start(out=wv_sb[:, kd, :], in_=wv_r[:, kd, :])

                for t in range(KT):
                    ps = ps1.tile([128, 512], F32, tag="ps1")
                    for kd in range(KD):
                        nc.tensor.matmul(
                            ps[:],
                            xT_sb[:, kd, ts(t, 128)],
                            wv_sb[:, kd, :],
                            start=(kd == 0),
                            stop=(kd == KD - 1),
                        )
                    nc.vector.tensor_add(
                        v_sb[:, t, :, 0:HD],
                        ps.rearrange("p (h c) -> p h c", c=HD),
                        bv_bc.rearrange("p (h c) -> p h c", c=HD),
                    )
                    nc.vector.tensor_scalar_mul(
                        v_sb[:, t, :, :], v_sb[:, t, :, :], pad_sb[:, t : t + 1]
                    )

            # ------- phase 1b + 2: per-pair q/k projection + attention -------
            # (interleaved: later pairs' projections fill PE bubbles while the
            # ScalarE-paced attention of earlier pairs runs)
            with (
                tc.tile_pool(name="wpool", bufs=2) as wpool,
                tc.tile_pool(name="epool", bufs=3) as epool,
                tc.tile_pool(name="ctxp", bufs=2) as ctxp,
                tc.tile_pool(name="octxp", bufs=3) as octxp,
                tc.tile_pool(name="rcp", bufs=3) as rcp,
                tc.tile_pool(name="psS", bufs=2, space="PSUM") as psS,
                tc.tile_pool(name="pvp", bufs=2, space="PSUM") as pvp,
                tc.tile_pool(name="trp", bufs=1, space="PSUM") as trp,
                tc.tile_pool(name="psQ", bufs=1, space="PSUM") as psQ,
            ):

                def qk_proj(m, w_r, tag, b_sb, o_sb):
                    w_sb = wpool.tile([128, KD, 128], F32R, tag=tag)
                    for kd in range(KD):
                        nc.gpsimd.dma_start(
                            out=w_sb[:, kd, :], in_=w_r[:, kd, ts(m, 128)]
                        )
                    for c in range(QC):
                        ps = psQ.tile([128, 512], F32, tag="psq")
                        for kd in range(KD):
                            nc.tensor.matmul(
                                ps[:],
                                w_sb[:, kd, :],
                                xT_sb[:, kd, ts(c, 512)],
                                start=(kd == 0),
                                stop=(kd == KD - 1),
                            )
                        nc.vector.tensor_scalar_add(
                            o_sb[:, m, ts(c, 512)], ps[:], b_sb[:, m : m + 1]
                        )

                for p in range(NPAIRS):
                    qk_proj(p, wq_r, "wq", bq_sb, qT_sb)
                    qk_proj(p, wk_r, "wk", bk_sb, kT_sb)
                    hA, hB = 2 * p, 2 * p + 1
                    for c in range(QC):
                        pvA = pvp.tile([128, 512], F32, tag="pv")
                        pvB = pvp.tile([128, 512], F32, tag="pv")
                        for kt in range(KT):
                            grp = psS.tile([128, 1024], F32, tag="grp")
                            # row-tiled concurrent pair: head A on array rows
                            # 0:64, head B on rows 64:128
                            nc.tensor.matmul(
                                grp[:, 0:512],
                                kT_sb[0:64, p, ts(kt, 128)],
                                qT_sb[0:64, p, ts(c, 512)],
                                start=True,
                                stop=True,
                            )
                            nc.tensor.matmul(
                                grp[:, 512:1024],
                                kT_sb[64:128, p, ts(kt, 128)],
                                qT_sb[64:128, p, ts(c, 512)],
                                start=True,
                                stop=True,
                            )
                            e = epool.tile([128, 1024], F32R, tag="e")
                            nc.scalar.activation(
                                e[:], grp[:], mybir.ActivationFunctionType.Exp,
                                scale=0.125,
                            )
                            nc.tensor.matmul(
                                pvA[0:65, :],
                                v_sb[:, kt, hA, :],
                                e[:, 0:512],
                                start=(kt == 0),
                                stop=(kt == KT - 1),
                            )
                            nc.tensor.matmul(
                                pvB[0:65, :],
                                v_sb[:, kt, hB, :],
                                e[:, 512:1024],
                                start=(kt == 0),
                                stop=(kt == KT - 1),
                            )
                        for head, pv in ((hA, pvA), (hB, pvB)):
                            ctxs = ctxp.tile([65, 512], F32, tag="ctxs")
                            nc.vector.tensor_copy(ctxs[:], pv[0:65, :])
                            tr = trp.tile([128, 512], F32, tag="tr")
                            octx = octxp.tile([128, 4, HD], F32, tag="octx")
                            rc = rcp.tile([128, 4], F32, tag="rc")
                            for blk in range(4):
                                nc.tensor.transpose(
                                    tr[:, ds(blk * 128, 65)],
                                    ctxs[:, ts(blk, 128)],
                                    ident[0:65, 0:65],
                                )
                                nc.vector.reciprocal(
                                    rc[:, blk : blk + 1],
                                    tr[:, ds(blk * 128 + HD, 1)],
                                )
                                nc.vector.tensor_scalar_mul(
                                    octx[:, blk, :],
                                    tr[:, ds(blk * 128, HD)],
                                    rc[:, blk : blk + 1],
                                )
                            nc.gpsimd.dma_start(
                                out=out_d[ds(c * 512, 512), ds(head * HD, HD)]
                                .rearrange("(blk p) d -> p blk d", p=128),
                                in_=octx[:],
                            )

    split_excess_waits(nc)
    return nc


_NC = None


def _get_nc():
    global _NC
    if _NC is None:
        _NC = build_nc()
    return _NC


def make_in_maps(hidden_states, pad, wq, bq, wk, bk, wv, bv):
    hidden_states = np.ascontiguousarray(np.asarray(hidden_states, dtype=np.float32))
    pad = np.asarray(pad, dtype=np.float32)
    in_maps = []
    for core in range(8):
        b, g = divmod(core, 2)
        sl = slice(512 * g, 512 * (g + 1))
        in_maps.append(
            {
                "xT": np.ascontiguousarray(hidden_states[b].T),
                "wqT": np.ascontiguousarray(np.asarray(wq, np.float32)[sl, :].T),
                "wkT": np.ascontiguousarray(np.asarray(wk, np.float32)[sl, :].T),
                "wvT": np.ascontiguousarray(np.asarray(wv, np.float32)[sl, :].T),
                "bq": np.ascontiguousarray(np.asarray(bq, np.float32)[sl]),
                "bk": np.ascontiguousarray(np.asarray(bk, np.float32)[sl]),
                "bv": np.ascontiguousarray(np.asarray(bv, np.float32)[sl]),
                "pad": np.ascontiguousarray(pad[b]),
            }
        )
    return in_maps


def assemble(results):
    out = np.empty((B, S, D), dtype=np.float32)
    for core in range(8):
        b, g = divmod(core, 2)
        out[b, :, 512 * g : 512 * (g + 1)] = results[core]["ctx"]
    return out


def kernel(hidden_states, pad, wq, bq, wk, bk, wv, bv):
    from concourse.bass_utils import run_bass_kernel_spmd

    nc = _get_nc()
    in_maps = make_in_maps(hidden_states, pad, wq, bq, wk, bk, wv, bv)
    res = run_bass_kernel_spmd(nc, in_maps, list(range(8)))
    return assemble(res.results)



# revision 6
# speedup vs baseline: 1.2926x; 1.2926x over previous
"""BertSelfAttention forward on 8 Trainium2 NeuronCores (Bass/Tile), v2.

Problem: B=4, S=2048, D=1024, H=16 heads, head_dim=64.
Sharding: 8 cores = (batch b in 0..4) x (head-group g in 0..2); each core
computes heads [8g, 8g+8) of batch b => output slice out[b, :, 512g:512(g+1)].

v2 design (vs the fp32r baseline):
  * All matmuls in bf16.  fp32r "HIGH" mode streams at ~2 cycles/row on HW;
    bf16 streams 1 column/cycle at the warm 2.4 GHz clock, halving PE time.
    Accumulation stays fp32 in PSUM; measured end-to-end rel err ~1e-3.
  * ScalarE exp is the pacing engine (33.5M exps/core at ~1.15us per
    [128,1024] ACTIVATE = ~294us busy).  The whole kernel is software-
    pipelined around it: per (pair, c) q-chunk, the PE issues score pairs
    two k-tiles ahead of the exps, PV matmuls right behind them, and fills
    its remaining slack with the next pair's q/k projection matmuls, so
    the ACT queue never starves and the PE never idles long enough to
    drop out of the warm HAM state.
  * pad mask is exact: exp(s + (1-pad)*-1e9) = exp(s)*pad, folded into the
    v rows and the denominator column (aug column 64 of v, scaled by pad).
  * Denominators ride the PV matmul (stationary v_aug [128, 65], col 64 =
    pad) and come out as row 64 of ctxT; PE-transpose 128-blocks, divide,
    DMA out.

PSUM (8 banks): score groups 2x[128,1024] (4) + pv/transpose ring 2x
[128,512] (2) + projection ring 2x[128,512] (2).
"""

import numpy as np
import ml_dtypes

import concourse.bass as bass
import concourse.tile as tile
from concourse import mybir
from concourse.bass import ds, ts
from concourse.masks import make_identity

B, S, D, H = 4, 2048, 1024, 16
HD = D // H          # 64
DH = 512             # head dims per core (8 heads)
NHEADS = 8           # heads per core
NPAIRS = 4           # head pairs per core
KT = 16              # k-tiles of 128 over S
QC = 4               # q-chunks of 512 over S
KD = 8               # contraction tiles of 128 over D

F32 = mybir.dt.float32
BF16 = mybir.dt.bfloat16

MAX_WAITS = 1


def split_excess_waits(nc):
    """This walrus build accepts only one sync-wait per instruction; hoist
    excess waits onto preceding NoOps on the same engine."""
    for f in nc.m.functions:
        for bb in f.blocks:
            insts = list(bb.instructions)
            out = []
            changed = False
            for inst in insts:
                si = inst.sync_info
                if si is not None and si.on_wait and len(si.on_wait) > MAX_WAITS:
                    waits = list(si.on_wait)
                    changed = True
                    k = 0
                    while len(waits) - k > MAX_WAITS:
                        nop = mybir.InstNoOp(
                            name=f"{inst.name}-ws{k}", engine=inst.engine
                        )
                        nop.sync_info = mybir.SyncInfo(
                            on_wait=waits[k : k + MAX_WAITS], on_update=[]
                        )
                        out.append(nop)
                        k += MAX_WAITS
                    si.on_wait = waits[k:]
                out.append(inst)
            if changed:
                bb.instructions = out
    return nc


def build_nc():
    nc = bass.Bass("TRN2", target_bir_lowering=False, debug=False, num_devices=8)

    xT_d = nc.declare_dram_parameter("xT", [D, S], BF16, isOutput=False)
    wqT_d = nc.declare_dram_parameter("wqT", [D, DH], BF16, isOutput=False)
    wkT_d = nc.declare_dram_parameter("wkT", [D, DH], BF16, isOutput=False)
    wvT_d = nc.declare_dram_parameter("wvT", [D, DH], BF16, isOutput=False)
    bq_d = nc.declare_dram_parameter("bq", [DH], F32, isOutput=False)
    bk_d = nc.declare_dram_parameter("bk", [DH], F32, isOutput=False)
    bv_d = nc.declare_dram_parameter("bv", [DH], F32, isOutput=False)
    pad_d = nc.declare_dram_parameter("pad", [S], F32, isOutput=False)
    out_d = nc.declare_dram_parameter("ctx", [S, DH], F32, isOutput=True)

    with tile.TileContext(nc) as tc:
        with (
            tc.tile_pool(name="persist", bufs=1) as persist,
            tc.tile_pool(name="small", bufs=1) as small,
        ):
            qT_sb = persist.tile([128, NPAIRS, S], BF16)
            kT_sb = persist.tile([128, NPAIRS, S], BF16)
            v_sb = persist.tile([128, KT, NHEADS, HD + 1], BF16)
            xT_sb = persist.tile([128, KD, S], BF16)
            wv_sb = persist.tile([128, KD, DH], BF16)
            wq_sb = persist.tile([128, KD, DH], BF16)
            wk_sb = persist.tile([128, KD, DH], BF16)
            bq_sb = small.tile([128, NPAIRS], F32)
            bk_sb = small.tile([128, NPAIRS], F32)
            pad_sb = small.tile([128, KT], F32)
            bv_bc = small.tile([128, DH], F32)
            ident = small.tile([128, 128], BF16)

            # small loads on gpsimd queue
            nc.gpsimd.dma_start(out=bq_sb, in_=bq_d.ap().rearrange("(m p) -> p m", p=128))
            nc.gpsimd.dma_start(out=bk_sb, in_=bk_d.ap().rearrange("(m p) -> p m", p=128))
            nc.gpsimd.dma_start(out=pad_sb, in_=pad_d.ap().rearrange("(t p) -> p t", p=128))
            nc.gpsimd.dma_start(
                out=bv_bc,
                in_=bass.AP(tensor=bv_d, offset=0, ap=[[0, 128], [1, DH]]),
            )
            make_identity(nc, ident[:])
            # col 64 of every (kt, head) slot stays 1.0 (denominator), *= pad
            nc.vector.memset(v_sb[:], 1.0)

            xT_r = xT_d.ap().rearrange("(t p) s -> p t s", p=128)
            wq_r = wqT_d.ap().rearrange("(t p) n -> p t n", p=128)
            wk_r = wkT_d.ap().rearrange("(t p) n -> p t n", p=128)
            wv_r = wvT_d.ap().rearrange("(t p) n -> p t n", p=128)

            # big loads on the sync-engine HWDGE queue, ordered so the
            # prologue's dependencies land first: wv, x(s0,s1), wk,
            # x(s2,s3), wq, x(s4..7)
            for kd in range(KD):
                nc.sync.dma_start(out=wv_sb[:, kd, :], in_=wv_r[:, kd, :])
            SCH = 8  # s-chunks of the xT load
            scw = S // SCH

            def load_x(j):
                nc.sync.dma_start(
                    out=xT_sb[:, :, ts(j, scw)], in_=xT_r[:, :, ts(j, scw)]
                )

            load_x(0)
            load_x(1)
            for kd in range(KD):
                nc.sync.dma_start(out=wk_sb[:, kd, :], in_=wk_r[:, kd, :])
            load_x(2)
            load_x(3)
            for kd in range(KD):
                nc.sync.dma_start(out=wq_sb[:, kd, :], in_=wq_r[:, kd, :])
            for j in range(4, SCH):
                load_x(j)

            with (
                tc.tile_pool(name="psS", bufs=2, space="PSUM") as psS,
                tc.tile_pool(name="pvp", bufs=2, space="PSUM") as pvp,
                tc.tile_pool(name="psQ", bufs=2, space="PSUM") as psQ,
                tc.tile_pool(name="epool", bufs=6) as epool,
                tc.tile_pool(name="ctxp", bufs=2) as ctxp,
                tc.tile_pool(name="octxp", bufs=3) as octxp,
                tc.tile_pool(name="rcp", bufs=3) as rcp,
            ):
                # ---- projection groups (8 accumulating MMs + evac) ----
                def vproj_group(t):
                    ps = psQ.tile([128, 512], F32, tag="psq")
                    for kd in range(KD):
                        nc.tensor.matmul(
                            ps[:],
                            xT_sb[:, kd, ts(t, 128)],
                            wv_sb[:, kd, :],
                            start=(kd == 0),
                            stop=(kd == KD - 1),
                        )
                    nc.vector.tensor_add(
                        v_sb[:, t, :, 0:HD],
                        ps.rearrange("p (h c) -> p h c", c=HD),
                        bv_bc.rearrange("p (h c) -> p h c", c=HD),
                    )
                    nc.vector.tensor_scalar_mul(
                        v_sb[:, t, :, :], v_sb[:, t, :, :], pad_sb[:, t : t + 1]
                    )

                def qkproj_group(w_sb, b_sb, o_sb, m, c):
                    ps = psQ.tile([128, 512], F32, tag="psq")
                    for kd in range(KD):
                        nc.tensor.matmul(
                            ps[:],
                            w_sb[:, kd, ts(m, 128)],
                            xT_sb[:, kd, ts(c, 512)],
                            start=(kd == 0),
                            stop=(kd == KD - 1),
                        )
                    nc.vector.tensor_scalar_add(
                        o_sb[:, m, ts(c, 512)], ps[:], b_sb[:, m : m + 1]
                    )

                # filler thunks per chunk, emitted inside the chunk's kt loop
                # to fill PE slack while ACT paces it.  Hard constraint: a
                # chunk's qT/kT inputs must be EMITTED during an earlier
                # chunk (PE executes in program order — a later-emitted
                # producer for an earlier-emitted consumer deadlocks).
                def qp(m, c):
                    return lambda: qkproj_group(wq_sb, bq_sb, qT_sb, m, c)

                def kp(m, c):
                    return lambda: qkproj_group(wk_sb, bk_sb, kT_sb, m, c)

                fb = [[] for _ in range(NPAIRS * QC)]
                # chunk 0 interleaves the remaining v-projections (vproj(t)
                # emitted at kt=t-4, ahead of its consumer PV(t))
                fb[0] = [(lambda t=t: vproj_group(t)) for t in range(4, KT)]
                fb[0].append(qp(0, 1))
                for p in range(NPAIRS):
                    base = QC * p
                    if p >= 1:
                        fb[base].append(qp(p, 1))
                    if p + 1 < NPAIRS:
                        fb[base + 1] += [qp(p, 2), kp(p + 1, 0), kp(p + 1, 1)]
                        fb[base + 2] += [qp(p, 3), kp(p + 1, 2), kp(p + 1, 3)]
                        fb[base + 3] += [qp(p + 1, 0)]
                    else:
                        fb[base + 1].append(qp(p, 2))
                        fb[base + 2].append(qp(p, 3))

                # ---- prologue PE work ----
                for t in range(4):
                    vproj_group(t)
                for c in range(QC):
                    qkproj_group(wk_sb, bk_sb, kT_sb, 0, c)
                qkproj_group(wq_sb, bq_sb, qT_sb, 0, 0)

                # ---- attention, flat-pipelined over 16 chunks ----
                def scores(p, c, kt):
                    grp = psS.tile([128, 1024], F32, tag="grp")
                    nc.tensor.matmul(
                        grp[:, 0:512],
                        kT_sb[0:64, p, ts(kt, 128)],
                        qT_sb[0:64, p, ts(c, 512)],
                        start=True,
                        stop=True,
                    )
                    nc.tensor.matmul(
                        grp[:, 512:1024],
                        kT_sb[64:128, p, ts(kt, 128)],
                        qT_sb[64:128, p, ts(c, 512)],
                        start=True,
                        stop=True,
                    )
                    return grp

                pending_tail = None

                for p in range(NPAIRS):
                    for c in range(QC):
                        chunk = QC * p + c
                        hA, hB = 2 * p, 2 * p + 1
                        grps = [scores(p, c, 0), scores(p, c, 1)]
                        if pending_tail is not None:
                            pending_tail()
                            pending_tail = None
                        pvA = pvp.tile([128, 512], F32, tag="pv")
                        pvB = pvp.tile([128, 512], F32, tag="pv")
                        fq = fb[chunk]
                        # consume chunk 0's fillers back-to-back from kt=0
                        # (vproj deadlines); later chunks spread theirs
                        slots = range(KT) if chunk == 0 else (2, 6, 10)
                        for kt in range(KT):
                            e = epool.tile([128, 1024], BF16, tag="e")
                            nc.scalar.activation(
                                e[:], grps[kt % 2][:],
                                mybir.ActivationFunctionType.Exp,
                                scale=0.125,
                            )
                            # filler projection work (PE slack)
                            if kt in slots and fq:
                                fq.pop(0)()
                            if kt + 2 < KT:
                                grps[kt % 2] = scores(p, c, kt + 2)
                            nc.tensor.matmul(
                                pvA[0:65, :],
                                v_sb[:, kt, hA, :],
                                e[:, 0:512],
                                start=(kt == 0),
                                stop=(kt == KT - 1),
                            )
                            nc.tensor.matmul(
                                pvB[0:65, :],
                                v_sb[:, kt, hB, :],
                                e[:, 512:1024],
                                start=(kt == 0),
                                stop=(kt == KT - 1),
                            )

                        def tail(p=p, c=c, hA=hA, hB=hB, pvA=pvA, pvB=pvB):
                            for head, pv in ((hA, pvA), (hB, pvB)):
                                ctxs = ctxp.tile([65, 512], BF16, tag="ctxs")
                                nc.vector.tensor_copy(ctxs[:], pv[0:65, :])
                                tr = pvp.tile([128, 512], BF16, tag="pv")
                                octx = octxp.tile([128, QC, HD], F32, tag="octx")
                                rc = rcp.tile([128, QC], F32, tag="rc")
                                for blk in range(4):
                                    nc.tensor.transpose(
                                        tr[:, ds(blk * 128, 65)],
                                        ctxs[:, ts(blk, 128)],
                                        ident[0:65, 0:65],
                                    )
                                    nc.vector.reciprocal(
                                        rc[:, blk : blk + 1],
                                        tr[:, ds(blk * 128 + HD, 1)],
                                    )
                                    nc.vector.tensor_scalar_mul(
                                        octx[:, blk, :],
                                        tr[:, ds(blk * 128, HD)],
                                        rc[:, blk : blk + 1],
                                    )
                                nc.gpsimd.dma_start(
                                    out=out_d[ds(c * 512, 512), ds(head * HD, HD)]
                                    .rearrange("(blk p) d -> p blk d", p=128),
                                    in_=octx[:],
                                )

                        pending_tail = tail

                if pending_tail is not None:
                    pending_tail()
                for fq in fb:  # safety: nothing should remain
                    for f in fq:
                        f()

    split_excess_waits(nc)
    return nc


_NC = None


def _get_nc():
    global _NC
    if _NC is None:
        _NC = build_nc()
    return _NC


def make_in_maps(hidden_states, pad, wq, bq, wk, bk, wv, bv):
    bf16 = ml_dtypes.bfloat16
    hidden_states = np.asarray(hidden_states, dtype=np.float32)
    pad = np.asarray(pad, dtype=np.float32)
    in_maps = []
    xT_b = [
        np.ascontiguousarray(hidden_states[b].T.astype(bf16)) for b in range(B)
    ]
    wq_t, wk_t, wv_t = (
        np.asarray(w, np.float32).astype(bf16) for w in (wq, wk, wv)
    )
    for core in range(8):
        b, g = divmod(core, 2)
        sl = slice(512 * g, 512 * (g + 1))
        in_maps.append(
            {
                "xT": xT_b[b],
                "wqT": np.ascontiguousarray(wq_t[sl, :].T),
                "wkT": np.ascontiguousarray(wk_t[sl, :].T),
                "wvT": np.ascontiguousarray(wv_t[sl, :].T),
                "bq": np.ascontiguousarray(np.asarray(bq, np.float32)[sl]),
                "bk": np.ascontiguousarray(np.asarray(bk, np.float32)[sl]),
                "bv": np.ascontiguousarray(np.asarray(bv, np.float32)[sl]),
                "pad": np.ascontiguousarray(pad[b]),
            }
        )
    return in_maps


def assemble(results):
    out = np.empty((B, S, D), dtype=np.float32)
    for core in range(8):
        b, g = divmod(core, 2)
        out[b, :, 512 * g : 512 * (g + 1)] = results[core]["ctx"]
    return out


def kernel(hidden_states, pad, wq, bq, wk, bk, wv, bv):
    from concourse.bass_utils import run_bass_kernel_spmd

    nc = _get_nc()
    in_maps = make_in_maps(hidden_states, pad, wq, bq, wk, bk, wv, bv)
    res = run_bass_kernel_spmd(nc, in_maps, list(range(8)))
    return assemble(res.results)


# revision 7
# speedup vs baseline: 1.3090x; 1.0127x over previous
"""BertSelfAttention forward on 8 Trainium2 NeuronCores (Bass/Tile), v3.

Problem: B=4, S=2048, D=1024, H=16 heads, head_dim=64.
Sharding: 8 cores = (batch b) x (head-group g); each core computes heads
[8g, 8g+8) of batch b => output slice out[b, :, 512g:512(g+1)].

Design (measured-informed):
  * All matmuls bf16 (fp32r HIGH streams ~2 cycles/row; bf16 1/row).
  * ScalarE exp paces the kernel (256 x [128,1024] ACTIVATEs ~287us busy).
    One flat software pipeline over all 256 (pair, q-chunk, k-tile)
    iterations: exp(i) runs while the PE issues scores(i+2), PV(i), and
    projection-filler groups, so ACT never waits at chunk boundaries.
  * Score pair: two K=64 row-tiles (heads on partition halves). rhs of the
    second tile reads a separate SBUF copy (qT2) - distinct stream source.
  * PV per head: K=128 keys x M=65 (v aug with pad column -> softmax
    denominators ride along as ctxT row 64).
  * Tail per chunk: DVE copies pv->SBUF bf16 right after the last PV;
    PE transposes + reciprocal scaling early in the next chunk.

PSUM (8 banks): score ring 2x[128,1024] (4) + pv/transpose ring
2x[128,512] (2) + projection ring 2x[128,512] (2).
"""

import numpy as np
import ml_dtypes

import concourse.bass as bass
import concourse.tile as tile
from concourse import mybir
from concourse.bass import ds, ts
from concourse.masks import make_identity

B, S, D, H = 4, 2048, 1024, 16
HD = D // H          # 64
DH = 512             # head dims per core (8 heads)
NHEADS = 8
NPAIRS = 4
KT = 16              # k-tiles of 128 over S
QC = 4               # q-chunks of 512 over S
KD = 8               # contraction tiles of 128 over D
NCHUNK = NPAIRS * QC
GI = NCHUNK * KT     # 256 global iterations

F32 = mybir.dt.float32
BF16 = mybir.dt.bfloat16

MAX_WAITS = 1


def split_excess_waits(nc):
    """This walrus build accepts only one sync-wait per instruction; hoist
    excess waits onto preceding NoOps on the same engine."""
    for f in nc.m.functions:
        for bb in f.blocks:
            insts = list(bb.instructions)
            out = []
            changed = False
            for inst in insts:
                si = inst.sync_info
                if si is not None and si.on_wait and len(si.on_wait) > MAX_WAITS:
                    waits = list(si.on_wait)
                    changed = True
                    k = 0
                    while len(waits) - k > MAX_WAITS:
                        nop = mybir.InstNoOp(
                            name=f"{inst.name}-ws{k}", engine=inst.engine
                        )
                        nop.sync_info = mybir.SyncInfo(
                            on_wait=waits[k : k + MAX_WAITS], on_update=[]
                        )
                        out.append(nop)
                        k += MAX_WAITS
                    si.on_wait = waits[k:]
                out.append(inst)
            if changed:
                bb.instructions = out
    return nc


def build_nc():
    nc = bass.Bass("TRN2", target_bir_lowering=False, debug=False, num_devices=8)

    xT_d = nc.declare_dram_parameter("xT", [D, S], BF16, isOutput=False)
    wqT_d = nc.declare_dram_parameter("wqT", [D, DH], BF16, isOutput=False)
    wkT_d = nc.declare_dram_parameter("wkT", [D, DH], BF16, isOutput=False)
    wvT_d = nc.declare_dram_parameter("wvT", [D, DH], BF16, isOutput=False)
    bq_d = nc.declare_dram_parameter("bq", [DH], F32, isOutput=False)
    bk_d = nc.declare_dram_parameter("bk", [DH], F32, isOutput=False)
    bv_d = nc.declare_dram_parameter("bv", [DH], F32, isOutput=False)
    pad_d = nc.declare_dram_parameter("pad", [S], F32, isOutput=False)
    out_d = nc.declare_dram_parameter("ctx", [S, DH], F32, isOutput=True)

    with tile.TileContext(nc) as tc:
        with (
            tc.tile_pool(name="persist", bufs=1) as persist,
            tc.tile_pool(name="small", bufs=1) as small,
        ):
            qT_sb = persist.tile([128, NPAIRS, S], BF16)
            qT2_sb = persist.tile([128, NPAIRS, S], BF16)
            kT_sb = persist.tile([128, NPAIRS, S], BF16)
            v_sb = persist.tile([128, KT, NHEADS, HD + 1], BF16)
            xT_sb = persist.tile([128, KD, S], BF16)
            wv_sb = persist.tile([128, KD, DH], BF16)
            wq_sb = persist.tile([128, KD, DH], BF16)
            wk_sb = persist.tile([128, KD, DH], BF16)
            bq_sb = small.tile([128, NPAIRS], F32)
            bk_sb = small.tile([128, NPAIRS], F32)
            pad_sb = small.tile([128, KT], F32)
            bv_bc = small.tile([128, DH], F32)
            ident = small.tile([128, 128], BF16)

            nc.gpsimd.dma_start(out=bq_sb, in_=bq_d.ap().rearrange("(m p) -> p m", p=128))
            nc.gpsimd.dma_start(out=bk_sb, in_=bk_d.ap().rearrange("(m p) -> p m", p=128))
            nc.gpsimd.dma_start(out=pad_sb, in_=pad_d.ap().rearrange("(t p) -> p t", p=128))
            nc.gpsimd.dma_start(
                out=bv_bc,
                in_=bass.AP(tensor=bv_d, offset=0, ap=[[0, 128], [1, DH]]),
            )
            make_identity(nc, ident[:])
            # col 64 of every (kt, head) slot stays 1.0 (denominator), *= pad
            nc.vector.memset(v_sb[:], 1.0)

            xT_r = xT_d.ap().rearrange("(t p) s -> p t s", p=128)
            wq_r = wqT_d.ap().rearrange("(t p) n -> p t n", p=128)
            wk_r = wkT_d.ap().rearrange("(t p) n -> p t n", p=128)
            wv_r = wvT_d.ap().rearrange("(t p) n -> p t n", p=128)

            # weights on the gpsimd queue, xT s-chunks on the sync HWDGE
            # queue - the two run concurrently
            nc.gpsimd.dma_start(out=wv_sb[:], in_=wv_r)
            nc.gpsimd.dma_start(out=wk_sb[:], in_=wk_r)
            nc.gpsimd.dma_start(out=wq_sb[:], in_=wq_r)
            SCH = 8
            scw = S // SCH
            for j in range(SCH):
                nc.sync.dma_start(
                    out=xT_sb[:, :, ts(j, scw)], in_=xT_r[:, :, ts(j, scw)]
                )

            with (
                tc.tile_pool(name="psS", bufs=2, space="PSUM") as psS,
                tc.tile_pool(name="pvp", bufs=2, space="PSUM") as pvp,
                tc.tile_pool(name="psQ", bufs=2, space="PSUM") as psQ,
                tc.tile_pool(name="epool", bufs=6) as epool,
                tc.tile_pool(name="ctxp", bufs=2) as ctxp,
                tc.tile_pool(name="octxp", bufs=3) as octxp,
                tc.tile_pool(name="rcp", bufs=3) as rcp,
            ):
                # ---- projection groups (8 accumulating MMs + evac) ----
                def vproj_group(t):
                    ps = psQ.tile([128, 512], F32, tag="psq")
                    for kd in range(KD):
                        nc.tensor.matmul(
                            ps[:],
                            xT_sb[:, kd, ts(t, 128)],
                            wv_sb[:, kd, :],
                            start=(kd == 0),
                            stop=(kd == KD - 1),
                        )
                    nc.vector.tensor_add(
                        v_sb[:, t, :, 0:HD],
                        ps.rearrange("p (h c) -> p h c", c=HD),
                        bv_bc.rearrange("p (h c) -> p h c", c=HD),
                    )
                    nc.vector.tensor_scalar_mul(
                        v_sb[:, t, :, :], v_sb[:, t, :, :], pad_sb[:, t : t + 1]
                    )

                def qkproj_group(w_sb, b_sb, o_sb, m, c):
                    ps = psQ.tile([128, 512], F32, tag="psq")
                    for kd in range(KD):
                        nc.tensor.matmul(
                            ps[:],
                            w_sb[:, kd, ts(m, 128)],
                            xT_sb[:, kd, ts(c, 512)],
                            start=(kd == 0),
                            stop=(kd == KD - 1),
                        )
                    nc.vector.tensor_scalar_add(
                        o_sb[:, m, ts(c, 512)], ps[:], b_sb[:, m : m + 1]
                    )
                    if o_sb is qT_sb:
                        # second stream source for the row-tiled score pair
                        nc.vector.tensor_copy(
                            qT2_sb[64:128, m, ts(c, 512)],
                            qT_sb[64:128, m, ts(c, 512)],
                        )

                def qp(m, c):
                    return lambda: qkproj_group(wq_sb, bq_sb, qT_sb, m, c)

                def kp(m, c):
                    return lambda: qkproj_group(wk_sb, bk_sb, kT_sb, m, c)

                # filler thunks per chunk (emission deadlines: a chunk's
                # qT/kT must be emitted before its first scores, which are
                # issued 2 iterations early in the previous chunk)
                fb = [[] for _ in range(NCHUNK)]
                fb[0] = [(lambda t=t: vproj_group(t)) for t in range(4, KT)]
                fb[0].append(qp(0, 1))
                for p in range(NPAIRS):
                    base = QC * p
                    if p >= 1:
                        fb[base].append(qp(p, 1))
                    if p + 1 < NPAIRS:
                        fb[base + 1] += [qp(p, 2), kp(p + 1, 0), kp(p + 1, 1)]
                        fb[base + 2] += [qp(p, 3), kp(p + 1, 2), kp(p + 1, 3)]
                        fb[base + 3] += [qp(p + 1, 0)]
                    else:
                        fb[base + 1].append(qp(p, 2))
                        fb[base + 2].append(qp(p, 3))

                # ---- prologue PE work ----
                for t in range(4):
                    vproj_group(t)
                for c in range(QC):
                    qkproj_group(wk_sb, bk_sb, kT_sb, 0, c)
                qkproj_group(wq_sb, bq_sb, qT_sb, 0, 0)

                def scores(gi):
                    ci, kt = divmod(gi, KT)
                    p, c = divmod(ci, QC)
                    grp = psS.tile([128, 1024], F32, tag="grp")
                    nc.tensor.matmul(
                        grp[:, 0:512],
                        kT_sb[0:64, p, ts(kt, 128)],
                        qT_sb[0:64, p, ts(c, 512)],
                        start=True,
                        stop=True,
                    )
                    nc.tensor.matmul(
                        grp[:, 512:1024],
                        kT_sb[64:128, p, ts(kt, 128)],
                        qT2_sb[64:128, p, ts(c, 512)],
                        start=True,
                        stop=True,
                    )
                    return grp

                # ---- flat global pipeline ----
                grps = {0: scores(0), 1: scores(1)}
                pvt = {}
                ctxt = {}
                pending_finish = None

                for gi in range(GI):
                    ci, kt = divmod(gi, KT)
                    p, c = divmod(ci, QC)
                    hA, hB = 2 * p, 2 * p + 1

                    e = epool.tile([128, 1024], BF16, tag="e")
                    nc.scalar.activation(
                        e[:], grps.pop(gi)[:],
                        mybir.ActivationFunctionType.Exp,
                        scale=0.125,
                    )
                    # filler projection work in PE slack
                    fq = fb[ci]
                    slots = range(13) if ci == 0 else (2, 6, 10)
                    if kt in slots and fq:
                        fq.pop(0)()
                    if gi + 2 < GI:
                        grps[gi + 2] = scores(gi + 2)
                    if kt == 0:
                        # finish previous chunk (PE transposes + DVE scale +
                        # DMA out) before this chunk's first PV
                        if pending_finish is not None:
                            pending_finish()
                            pending_finish = None
                        pvt[ci] = (
                            pvp.tile([128, 512], F32, tag="pv", name="pvA"),
                            pvp.tile([128, 512], F32, tag="pv", name="pvB"),
                        )
                    pvA, pvB = pvt[ci]
                    nc.tensor.matmul(
                        pvA[0:65, :],
                        v_sb[:, kt, hA, :],
                        e[:, 0:512],
                        start=(kt == 0),
                        stop=(kt == KT - 1),
                    )
                    nc.tensor.matmul(
                        pvB[0:65, :],
                        v_sb[:, kt, hB, :],
                        e[:, 512:1024],
                        start=(kt == 0),
                        stop=(kt == KT - 1),
                    )

                    if kt == KT - 1:
                        # evacuate pv PSUM now (frees the ring for the next
                        # chunk); the rest of the tail runs at next kt==0
                        ctxsA = ctxp.tile([65, 512], BF16, tag="ctxs", name="ctxsA")
                        ctxsB = ctxp.tile([65, 512], BF16, tag="ctxs", name="ctxsB")
                        nc.vector.tensor_copy(ctxsA[:], pvA[0:65, :])
                        nc.vector.tensor_copy(ctxsB[:], pvB[0:65, :])
                        ctxt[ci] = (ctxsA, ctxsB)
                        del pvt[ci]

                        def finish(p=p, c=c, hA=hA, hB=hB, ci=ci):
                            for head, ctxs in zip((hA, hB), ctxt.pop(ci)):
                                tr = pvp.tile([128, 512], BF16, tag="pv", name="tr")
                                octx = octxp.tile([128, QC, HD], F32, tag="octx")
                                rc = rcp.tile([128, QC], F32, tag="rc")
                                for blk in range(4):
                                    nc.tensor.transpose(
                                        tr[:, ds(blk * 128, 65)],
                                        ctxs[:, ts(blk, 128)],
                                        ident[0:65, 0:65],
                                    )
                                    nc.vector.reciprocal(
                                        rc[:, blk : blk + 1],
                                        tr[:, ds(blk * 128 + HD, 1)],
                                    )
                                    nc.vector.tensor_scalar_mul(
                                        octx[:, blk, :],
                                        tr[:, ds(blk * 128, HD)],
                                        rc[:, blk : blk + 1],
                                    )
                                nc.gpsimd.dma_start(
                                    out=out_d[ds(c * 512, 512), ds(head * HD, HD)]
                                    .rearrange("(blk p) d -> p blk d", p=128),
                                    in_=octx[:],
                                )

                        pending_finish = finish

                if pending_finish is not None:
                    pending_finish()
                for fq in fb:  # safety: nothing should remain
                    for f in fq:
                        f()

    split_excess_waits(nc)
    return nc


_NC = None


def _get_nc():
    global _NC
    if _NC is None:
        _NC = build_nc()
    return _NC


def make_in_maps(hidden_states, pad, wq, bq, wk, bk, wv, bv):
    bf16 = ml_dtypes.bfloat16
    hidden_states = np.asarray(hidden_states, dtype=np.float32)
    pad = np.asarray(pad, dtype=np.float32)
    in_maps = []
    xT_b = [
        np.ascontiguousarray(hidden_states[b].T.astype(bf16)) for b in range(B)
    ]
    wq_t, wk_t, wv_t = (
        np.asarray(w, np.float32).astype(bf16) for w in (wq, wk, wv)
    )
    for core in range(8):
        b, g = divmod(core, 2)
        sl = slice(512 * g, 512 * (g + 1))
        in_maps.append(
            {
                "xT": xT_b[b],
                "wqT": np.ascontiguousarray(wq_t[sl, :].T),
                "wkT": np.ascontiguousarray(wk_t[sl, :].T),
                "wvT": np.ascontiguousarray(wv_t[sl, :].T),
                "bq": np.ascontiguousarray(np.asarray(bq, np.float32)[sl]),
                "bk": np.ascontiguousarray(np.asarray(bk, np.float32)[sl]),
                "bv": np.ascontiguousarray(np.asarray(bv, np.float32)[sl]),
                "pad": np.ascontiguousarray(pad[b]),
            }
        )
    return in_maps


def assemble(results):
    out = np.empty((B, S, D), dtype=np.float32)
    for core in range(8):
        b, g = divmod(core, 2)
        out[b, :, 512 * g : 512 * (g + 1)] = results[core]["ctx"]
    return out


def kernel(hidden_states, pad, wq, bq, wk, bk, wv, bv):
    from concourse.bass_utils import run_bass_kernel_spmd

    nc = _get_nc()
    in_maps = make_in_maps(hidden_states, pad, wq, bq, wk, bk, wv, bv)
    res = run_bass_kernel_spmd(nc, in_maps, list(range(8)))
    return assemble(res.results)


# revision 12
# speedup vs baseline: 1.4004x; 1.0698x over previous
"""BertSelfAttention forward on 8 Trainium2 NeuronCores (Bass/Tile), v4.

Problem: B=4, S=2048, D=1024, H=16 heads, head_dim=64.
Sharding: 8 cores = (batch b) x (head-group g); each core computes heads
[8g, 8g+8) of batch b => output slice out[b, :, 512g:512(g+1)].

Design (measurement-driven):
  * All matmuls bf16 (fp32r HIGH streams ~2 cycles/row; bf16 1 col/cycle).
  * One flat software pipeline over all 256 (pair, q-chunk, k-tile)
    iterations: per iteration the PE issues the score pair for i+2, the
    two PV matmuls for i, and a 2-matmul slice of a projection group, so
    both the PE (~89% busy, the bottleneck) and ScalarE exp (~72%) stay
    fed with no bursts.
  * Score pair: two K=64 row-tiles (heads on partition halves of qT/kT).
  * PV per head: K=128 keys x M=65 (v augmented with the pad column ->
    softmax denominators come out as ctxT row 64).
  * Chunk tail: DVE copies pv->SBUF bf16; a single hardware XBAR
    dma_start_transpose turns [80,512] ctxT into [128,4,80] q-major; DVE
    reciprocal+scale; DMA out.  No PE transposes at all.
  * Ramp: xT s-chunks split across the sync and scalar HWDGE queues in
    parallel with the weights on the gpsimd queue.

PSUM (8 banks): score ring 2x[128,1024] (4) + pv ring 2x[128,512] (2) +
projection ring 2x[128,512] (2).
"""

import numpy as np
import ml_dtypes

import concourse.bass as bass
import concourse.tile as tile
from concourse import mybir
from concourse.bass import ds, ts

B, S, D, H = 4, 2048, 1024, 16
HD = D // H          # 64
DH = 512             # head dims per core (8 heads)
NHEADS = 8
NPAIRS = 4
KT = 16              # k-tiles of 128 over S
QC = 4               # q-chunks of 512 over S
KD = 8               # contraction tiles of 128 over D
NCHUNK = NPAIRS * QC
GI = NCHUNK * KT     # 256 global iterations
TRW = 80             # transpose rows: 65 used, padded to a 16-multiple

F32 = mybir.dt.float32
BF16 = mybir.dt.bfloat16

MAX_WAITS = 1


def split_excess_waits(nc):
    """This walrus build accepts only one sync-wait per instruction; hoist
    excess waits onto preceding NoOps on the same engine."""
    for f in nc.m.functions:
        for bb in f.blocks:
            insts = list(bb.instructions)
            out = []
            changed = False
            for inst in insts:
                si = inst.sync_info
                if si is not None and si.on_wait and len(si.on_wait) > MAX_WAITS:
                    waits = list(si.on_wait)
                    changed = True
                    k = 0
                    while len(waits) - k > MAX_WAITS:
                        nop = mybir.InstNoOp(
                            name=f"{inst.name}-ws{k}", engine=inst.engine
                        )
                        nop.sync_info = mybir.SyncInfo(
                            on_wait=waits[k : k + MAX_WAITS], on_update=[]
                        )
                        out.append(nop)
                        k += MAX_WAITS
                    si.on_wait = waits[k:]
                out.append(inst)
            if changed:
                bb.instructions = out
    return nc


def build_nc():
    nc = bass.Bass("TRN2", target_bir_lowering=False, debug=False, num_devices=8)

    xT_d = nc.declare_dram_parameter("xT", [D, S], BF16, isOutput=False)
    wqT_d = nc.declare_dram_parameter("wqT", [D, DH], BF16, isOutput=False)
    wkT_d = nc.declare_dram_parameter("wkT", [D, DH], BF16, isOutput=False)
    wvT_d = nc.declare_dram_parameter("wvT", [D, DH], BF16, isOutput=False)
    bq_d = nc.declare_dram_parameter("bq", [DH], F32, isOutput=False)
    bk_d = nc.declare_dram_parameter("bk", [DH], F32, isOutput=False)
    bv_d = nc.declare_dram_parameter("bv", [DH], F32, isOutput=False)
    pad_d = nc.declare_dram_parameter("pad", [S], F32, isOutput=False)
    out_d = nc.declare_dram_parameter("ctx", [S, DH], F32, isOutput=True)

    with tile.TileContext(nc) as tc:
        with (
            tc.tile_pool(name="persist", bufs=1) as persist,
            tc.tile_pool(name="small", bufs=1) as small,
        ):
            qT_sb = persist.tile([128, NPAIRS, S], BF16)
            kT_sb = persist.tile([128, NPAIRS, S], BF16)
            v_sb = persist.tile([128, KT, NHEADS, HD + 1], BF16)
            xT_sb = persist.tile([128, KD, S], BF16)
            wv_sb = persist.tile([128, KD, DH], BF16)
            wq_sb = persist.tile([128, KD, DH], BF16)
            wk_sb = persist.tile([128, KD, DH], BF16)
            bq_sb = small.tile([128, NPAIRS], F32)
            bk_sb = small.tile([128, NPAIRS], F32)
            pad_sb = small.tile([128, KT], F32)
            bv_bc = small.tile([128, DH], F32)

            nc.gpsimd.dma_start(out=bq_sb, in_=bq_d.ap().rearrange("(m p) -> p m", p=128))
            nc.gpsimd.dma_start(out=bk_sb, in_=bk_d.ap().rearrange("(m p) -> p m", p=128))
            nc.gpsimd.dma_start(out=pad_sb, in_=pad_d.ap().rearrange("(t p) -> p t", p=128))
            nc.gpsimd.dma_start(
                out=bv_bc,
                in_=bass.AP(tensor=bv_d, offset=0, ap=[[0, 128], [1, DH]]),
            )
            # col 64 of every (kt, head) slot stays 1.0 (denominator), *= pad
            nc.vector.memset(v_sb[:], 1.0)

            xT_r = xT_d.ap().rearrange("(t p) s -> p t s", p=128)
            wq_r = wqT_d.ap().rearrange("(t p) n -> p t n", p=128)
            wk_r = wkT_d.ap().rearrange("(t p) n -> p t n", p=128)
            wv_r = wvT_d.ap().rearrange("(t p) n -> p t n", p=128)

            # weights on the gpsimd queue; xT s-chunks split across the two
            # HWDGE queues (sync + scalar) - all three run concurrently
            nc.gpsimd.dma_start(out=wv_sb[:], in_=wv_r)
            nc.gpsimd.dma_start(out=wk_sb[:], in_=wk_r)
            nc.gpsimd.dma_start(out=wq_sb[:], in_=wq_r)
            SCH = 8
            scw = S // SCH
            for j in range(SCH):
                eng = nc.sync if j < 4 else nc.scalar
                eng.dma_start(
                    out=xT_sb[:, :, ts(j, scw)], in_=xT_r[:, :, ts(j, scw)]
                )

            with (
                tc.tile_pool(name="psS", bufs=2, space="PSUM") as psS,
                tc.tile_pool(name="pvp", bufs=2, space="PSUM") as pvp,
                tc.tile_pool(name="psQ", bufs=2, space="PSUM") as psQ,
                tc.tile_pool(name="epool", bufs=6) as epool,
                tc.tile_pool(name="ctxp", bufs=2) as ctxp,
                tc.tile_pool(name="trp", bufs=3) as trp,
                tc.tile_pool(name="octxp", bufs=3) as octxp,
                tc.tile_pool(name="rcp", bufs=3) as rcp,
            ):
                # ---- projection groups (8 accumulating MMs + evac) ----
                def vproj_group(t):
                    ps = psQ.tile([128, 512], F32, tag="psq")
                    for kd in range(KD):
                        nc.tensor.matmul(
                            ps[:],
                            xT_sb[:, kd, ts(t, 128)],
                            wv_sb[:, kd, :],
                            start=(kd == 0),
                            stop=(kd == KD - 1),
                        )
                    nc.vector.tensor_add(
                        v_sb[:, t, :, 0:HD],
                        ps.rearrange("p (h c) -> p h c", c=HD),
                        bv_bc.rearrange("p (h c) -> p h c", c=HD),
                    )
                    nc.vector.tensor_scalar_mul(
                        v_sb[:, t, :, :], v_sb[:, t, :, :], pad_sb[:, t : t + 1]
                    )

                def qkproj_group(w_sb, b_sb, o_sb, m, c):
                    ps = psQ.tile([128, 512], F32, tag="psq")
                    for kd in range(KD):
                        nc.tensor.matmul(
                            ps[:],
                            w_sb[:, kd, ts(m, 128)],
                            xT_sb[:, kd, ts(c, 512)],
                            start=(kd == 0),
                            stop=(kd == KD - 1),
                        )
                    nc.vector.tensor_scalar_add(
                        o_sb[:, m, ts(c, 512)], ps[:], b_sb[:, m : m + 1]
                    )

                def qkproj_micros(w_sb, b_sb, o_sb, m, c):
                    """The same group as 4 micro-thunks of 2 matmuls each,
                    consumed one per pipeline iteration to avoid PE bursts."""
                    state = {}

                    def mic(j):
                        def run():
                            if j == 0:
                                state["ps"] = psQ.tile(
                                    [128, 512], F32, tag="psq", name="psqm"
                                )
                            ps = state["ps"]
                            for kd in (2 * j, 2 * j + 1):
                                nc.tensor.matmul(
                                    ps[:],
                                    w_sb[:, kd, ts(m, 128)],
                                    xT_sb[:, kd, ts(c, 512)],
                                    start=(kd == 0),
                                    stop=(kd == KD - 1),
                                )
                            if j == 3:
                                nc.vector.tensor_scalar_add(
                                    o_sb[:, m, ts(c, 512)], ps[:], b_sb[:, m : m + 1]
                                )

                        return run

                    return [mic(j) for j in range(4)]

                def qp(m, c):
                    return qkproj_micros(wq_sb, bq_sb, qT_sb, m, c)

                def kp(m, c):
                    return qkproj_micros(wk_sb, bk_sb, kT_sb, m, c)

                # filler micro-thunks per chunk (deadlines: a chunk's qT/kT
                # groups must be fully emitted before its first scores,
                # which are issued 2 iterations early in the prior chunk)
                fb = [[] for _ in range(NCHUNK)]
                # chunk 0 is coarse: vproj(t) at kt=t-4, then the whole
                # qT(0,1) group at kt=12 (its consumer scores(16) is
                # emitted at kt=14 - micros would land too late)
                fb[0] = [(lambda t=t: vproj_group(t)) for t in range(4, KT)]
                fb[0].append(lambda: qkproj_group(wq_sb, bq_sb, qT_sb, 0, 1))
                for p in range(NPAIRS):
                    base = QC * p
                    if p >= 1:
                        fb[base] += qp(p, 1)
                    if p + 1 < NPAIRS:
                        fb[base + 1] += qp(p, 2) + kp(p + 1, 0) + kp(p + 1, 1)
                        fb[base + 2] += qp(p, 3) + kp(p + 1, 2) + kp(p + 1, 3)
                        fb[base + 3] += qp(p + 1, 0)
                    else:
                        fb[base + 1] += qp(p, 2)
                        fb[base + 2] += qp(p, 3)

                # ---- prologue PE work ----
                for t in range(4):
                    vproj_group(t)
                for c in range(QC):
                    qkproj_group(wk_sb, bk_sb, kT_sb, 0, c)
                qkproj_group(wq_sb, bq_sb, qT_sb, 0, 0)

                def scores(gi):
                    ci, kt = divmod(gi, KT)
                    p, c = divmod(ci, QC)
                    grp = psS.tile([128, 1024], F32, tag="grp")
                    nc.tensor.matmul(
                        grp[:, 0:512],
                        kT_sb[0:64, p, ts(kt, 128)],
                        qT_sb[0:64, p, ts(c, 512)],
                        start=True,
                        stop=True,
                    )
                    nc.tensor.matmul(
                        grp[:, 512:1024],
                        kT_sb[64:128, p, ts(kt, 128)],
                        qT_sb[64:128, p, ts(c, 512)],
                        start=True,
                        stop=True,
                    )
                    return grp

                # ---- flat global pipeline ----
                grps = {0: scores(0), 1: scores(1)}
                pvt = {}
                ctxt = {}
                pending_finish = None

                for gi in range(GI):
                    ci, kt = divmod(gi, KT)
                    p, c = divmod(ci, QC)
                    hA, hB = 2 * p, 2 * p + 1

                    e = epool.tile([128, 1024], BF16, tag="e")
                    nc.scalar.activation(
                        e[:], grps.pop(gi)[:],
                        mybir.ActivationFunctionType.Exp,
                        scale=0.125,
                    )
                    # filler projection work in PE slack (one micro/whole
                    # thunk per iteration, FIFO)
                    fq = fb[ci]
                    if fq and (ci > 0 or kt < 13):
                        fq.pop(0)()
                    if gi + 2 < GI:
                        grps[gi + 2] = scores(gi + 2)
                    if kt == 0:
                        if pending_finish is not None:
                            pending_finish()
                            pending_finish = None
                        pvt[ci] = (
                            pvp.tile([128, 512], F32, tag="pv", name="pvA"),
                            pvp.tile([128, 512], F32, tag="pv", name="pvB"),
                        )
                    pvA, pvB = pvt[ci]
                    nc.tensor.matmul(
                        pvA[0:65, :],
                        v_sb[:, kt, hA, :],
                        e[:, 0:512],
                        start=(kt == 0),
                        stop=(kt == KT - 1),
                    )
                    nc.tensor.matmul(
                        pvB[0:65, :],
                        v_sb[:, kt, hB, :],
                        e[:, 512:1024],
                        start=(kt == 0),
                        stop=(kt == KT - 1),
                    )

                    if kt == KT - 1:
                        ctxsA = ctxp.tile([TRW, 512], BF16, tag="ctxs", name="ctxsA")
                        ctxsB = ctxp.tile([TRW, 512], BF16, tag="ctxs", name="ctxsB")
                        nc.vector.tensor_copy(ctxsA[0:65, :], pvA[0:65, :])
                        nc.vector.tensor_copy(ctxsB[0:65, :], pvB[0:65, :])
                        ctxt[ci] = (ctxsA, ctxsB)
                        del pvt[ci]

                        def finish(p=p, c=c, hA=hA, hB=hB, ci=ci):
                            for head, ctxs in zip((hA, hB), ctxt.pop(ci)):
                                trT = trp.tile([128, QC, TRW], BF16, tag="trT")
                                nc.sync.dma_start_transpose(trT[:], ctxs[:])
                                octx = octxp.tile([128, QC, HD], F32, tag="octx")
                                rc = rcp.tile([128, QC], F32, tag="rc")
                                nc.vector.reciprocal(rc[:, :], trT[:, :, HD])
                                for blk in range(4):
                                    nc.vector.tensor_scalar_mul(
                                        octx[:, blk, :],
                                        trT[:, blk, 0:HD],
                                        rc[:, blk : blk + 1],
                                    )
                                nc.gpsimd.dma_start(
                                    out=out_d[ds(c * 512, 512), ds(head * HD, HD)]
                                    .rearrange("(blk p) d -> p blk d", p=128),
                                    in_=octx[:],
                                )

                        pending_finish = finish

                if pending_finish is not None:
                    pending_finish()
                for fq in fb:  # safety: nothing should remain
                    for f in fq:
                        f()

    split_excess_waits(nc)
    return nc


_NC = None


def _get_nc():
    global _NC
    if _NC is None:
        _NC = build_nc()
    return _NC


def make_in_maps(hidden_states, pad, wq, bq, wk, bk, wv, bv):
    bf16 = ml_dtypes.bfloat16
    hidden_states = np.asarray(hidden_states, dtype=np.float32)
    pad = np.asarray(pad, dtype=np.float32)
    in_maps = []
    xT_b = [
        np.ascontiguousarray(hidden_states[b].T.astype(bf16)) for b in range(B)
    ]
    wq_t, wk_t, wv_t = (
        np.asarray(w, np.float32).astype(bf16) for w in (wq, wk, wv)
    )
    for core in range(8):
        b, g = divmod(core, 2)
        sl = slice(512 * g, 512 * (g + 1))
        in_maps.append(
            {
                "xT": xT_b[b],
                "wqT": np.ascontiguousarray(wq_t[sl, :].T),
                "wkT": np.ascontiguousarray(wk_t[sl, :].T),
                "wvT": np.ascontiguousarray(wv_t[sl, :].T),
                "bq": np.ascontiguousarray(np.asarray(bq, np.float32)[sl]),
                "bk": np.ascontiguousarray(np.asarray(bk, np.float32)[sl]),
                "bv": np.ascontiguousarray(np.asarray(bv, np.float32)[sl]),
                "pad": np.ascontiguousarray(pad[b]),
            }
        )
    return in_maps


def assemble(results):
    out = np.empty((B, S, D), dtype=np.float32)
    for core in range(8):
        b, g = divmod(core, 2)
        out[b, :, 512 * g : 512 * (g + 1)] = results[core]["ctx"]
    return out


def kernel(hidden_states, pad, wq, bq, wk, bk, wv, bv):
    from concourse.bass_utils import run_bass_kernel_spmd

    nc = _get_nc()
    in_maps = make_in_maps(hidden_states, pad, wq, bq, wk, bk, wv, bv)
    res = run_bass_kernel_spmd(nc, in_maps, list(range(8)))
    return assemble(res.results)
